# revision 2
# baseline (speedup 1.0000x reference)
"""Trainium2 Bass kernel v2 for nn_BCA_17274358465235.

Key changes vs baseline:
- exp split across ACT (table exp -> bf16) and DVE (Schraudolph int16
  bit-trick -> bitcast bf16), removing the single-engine softmax wall.
- sim pairing (key chunk p with p+16): fy2p holds image-half key groups
  in partition groups 0-63 / 64-127, so no fy duplication copy and the
  bilinear upsample runs on all 128 partitions (half the DVE time).
- xk holds both image halves (blocks interleaved top0,bot0,top1,...)
  so the fself stream consumes in pv-pair order with no stalls; global
  key indexing keeps sim/fself consistent on every core.
- biases folded into matmuls via ones-rows (h1y_aug/h1x_aug), so all
  PSUM->SBUF projection moves are plain ACT scaled copies.
- half-0 softmax tail + up-projection + output DMA overlap half-1
  attention.
"""
import sys

for _p in ("/opt/pypackages", "/opt/trn_rl_repo"):
    if _p not in sys.path:
        sys.path.insert(0, _p)

import numpy as np

import concourse.bacc as bacc
import concourse.mybir as mybir
import concourse.tile as tile
from concourse.bass_utils import run_bass_kernel_spmd

F32 = mybir.dt.float32
F32R = mybir.dt.float32r
BF16 = mybir.dt.bfloat16
I16 = mybir.dt.int16
F16 = mybir.dt.float16
EXP = mybir.ActivationFunctionType.Exp
COPY = mybir.ActivationFunctionType.Copy
IDENT = mybir.ActivationFunctionType.Identity
MUL, ADD = mybir.AluOpType.mult, mybir.AluOpType.add

B, CX, CY, CM = 4, 256, 512, 64
H = W = 64
HY = WY = 32
N = H * W
NH = N // 2

# Schraudolph exp in bf16-bit space: round(x*A + Bc) as int16, bitcast bf16
SCH_A = 128.0 / float(np.log(2.0))
SCH_B = 127.0 * 128.0 - 0.5 - 0.043 * 128.0

# exp-engine schedule: True -> ACT, False -> DVE (Schraudolph)
R0_ACT = 17   # of 32 units in half 0
R1_ACT = 19   # of 32 units in half 1


def _act_sched(r):
    return [((i + 1) * r) // 32 > (i * r) // 32 for i in range(32)]


_CACHE = {}


def _build(debug=False):
    nc = bacc.Bacc("TRN2", target_bir_lowering=False, debug=False,
                   enable_asserts=False)

    xl = nc.dram_tensor("xl", [128, 4096], F16, kind="ExternalInput").ap()
    xk = nc.dram_tensor("xk", [128, 8192], F16, kind="ExternalInput").ap()
    yb = nc.dram_tensor("yb", [128, 4096], F16, kind="ExternalInput").ap()
    wpack = nc.dram_tensor("wpack", [128, 1666], F16, kind="ExternalInput").ap()
    wb32 = nc.dram_tensor("wb32", [128, 2], F32, kind="ExternalInput").ap()
    out = nc.dram_tensor("out", [128, 4096], F32, kind="ExternalOutput").ap()
    if debug:
        d_fy2p = nc.dram_tensor("d_fy2p", [128, 2048], F32, kind="ExternalOutput").ap()
        d_fx2 = nc.dram_tensor("d_fx2", [128, 2048], F32, kind="ExternalOutput").ap()
        d_h1s = nc.dram_tensor("d_h1s", [65, 4096], F32, kind="ExternalOutput").ap()
        d_sim0 = nc.dram_tensor("d_sim0", [128, 1024], F32, kind="ExternalOutput").ap()
        d_scaled = nc.dram_tensor("d_scaled", [65, 2048], F32, kind="ExternalOutput").ap()
        d_et0 = nc.dram_tensor("d_et0", [128, 1024], F32, kind="ExternalOutput").ap()
        d_fout = nc.dram_tensor("d_fout", [65, 2048], F32, kind="ExternalOutput").ap()
        d_fself = nc.dram_tensor("d_fself", [128, 2080], F32, kind="ExternalOutput").ap()

    sched0 = _act_sched(R0_ACT)
    sched1 = _act_sched(R1_ACT)

    with tile.TileContext(nc) as tc:
        with tc.tile_pool(name="sbW", bufs=1) as sbW, \
             tc.tile_pool(name="sbM", bufs=1) as sbM:
            # ---- long-lived SBUF ----
            t_xl = sbM.tile([128, 4096], F16)
            t_xk = sbM.tile([128, 8192], F16)
            t_yb = sbM.tile([128, 4096], F16)
            fy2p = sbM.tile([128, 2048], F16)
            fx2 = sbM.tile([128, 2048], F16)
            fselfT = sbM.tile([128, 65 * 32], BF16)
            scaled = sbM.tile([65, 2048], F16)
            out_sb = sbM.tile([128, 4096], F32)

            t_wpack = sbW.tile([128, 1666], F16)
            t_ws12 = t_wpack[:, 0:128]          # fused Ws2@Ws1 (2 cin chunks)
            t_wx12 = t_wpack[:, 128:384]        # fused [Wx12|Wx12] per chunk
            t_wy12 = t_wpack[:, 384:1408]       # fused Wy12/16, zero-padded
            t_wupt = t_wpack[0:65, 1408:1664]
            t_wb32 = sbW.tile([128, 2], F32)
            t_bx2d = t_wb32[:, 0:1]
            t_by16 = t_wb32[:, 1:2]

            # ================= phase 1: DMAs + projections ============
            sbP_cm = tc.tile_pool(name="sbP", bufs=1)
            sbP = sbP_cm.__enter__()
            psP_cm = tc.tile_pool(name="psP", bufs=1, space="PSUM")
            psP = psP_cm.__enter__()

            # input DMAs in arrival-priority order
            nc.sync.dma_start(t_wpack[:], wpack[:])
            nc.sync.dma_start(t_wb32[:], wb32[:])
            nc.sync.dma_start(t_yb[:, 0:2048], yb[:, 0:2048])
            nc.sync.dma_start(t_yb[:, 2048:4096], yb[:, 2048:4096])
            # arrival priority: half-0 queries (xl-a), first fself pair
            # group (xkA), then the rest
            nc.sync.dma_start(t_xl[:, 0:2048], xl[:, 0:2048])
            nc.sync.dma_start(t_xk[:, 0:2048], xk[:, 0:2048])
            nc.sync.dma_start(t_xk[:, 2048:4096], xk[:, 2048:4096])
            nc.sync.dma_start(t_xl[:, 2048:4096], xl[:, 2048:4096])
            nc.sync.dma_start(t_xk[:, 4096:8192], xk[:, 4096:8192])

            # warm the ACT exp table
            t_dum = sbP.tile([1, 32], F32)
            nc.vector.memset(t_dum[:], 0.0)
            t_dum2 = sbP.tile([1, 32], F32)
            nc.scalar.activation(t_dum2[:], t_dum[:], EXP)
            # warm the PE p-state with full-array dummy matmuls (junk)
            t_wrm = sbP.tile([128, 640], F16, name="t_wrm")
            nc.vector.memset(t_wrm[:], 0.0)
            p_wrm = psP.tile([128, 512], F32, tag="pb", bufs=2, name="p_wrm")
            for wi in range(14):
                nc.tensor.matmul(p_wrm[:], t_wrm[:, 0:128],
                                 t_wrm[:, 128:640], start=True, stop=True)

            # ---- fy path: fyc' = (Wy12 @ y + by)/16 directly ----
            # p_main [128, 544]: cols 0:512 = 16 coarse-row slots per group
            # (g0 rows 0-15, g1 rows 16-31), cols 512:544 = edge slot
            # (g0 row 16, g1 row 15).  Zero-padded lhsT halves write both
            # partition groups from different token ranges.
            p_main = psP.tile([128, 544], F32, name="p_main")
            for a in range(4):
                lo = t_wy12[:, (2 * a) * 128:(2 * a) * 128 + 128]
                hi = t_wy12[:, (2 * a + 1) * 128:(2 * a + 1) * 128 + 128]
                nc.tensor.matmul(p_main[:, 0:512], lo,
                                 t_yb[:, a * 512:a * 512 + 512],
                                 start=(a == 0), stop=False)
                nc.tensor.matmul(p_main[:, 0:512], hi,
                                 t_yb[:, 2048 + a * 512:2048 + a * 512 + 512],
                                 start=False, stop=(a == 3))
            for a in range(4):
                lo = t_wy12[:, (2 * a) * 128:(2 * a) * 128 + 128]
                hi = t_wy12[:, (2 * a + 1) * 128:(2 * a + 1) * 128 + 128]
                nc.tensor.matmul(p_main[:, 512:544], lo,
                                 t_yb[:, 2048 + a * 512:2048 + a * 512 + 32],
                                 start=(a == 0), stop=False)
                nc.tensor.matmul(p_main[:, 512:544], hi,
                                 t_yb[:, a * 512 + 480:a * 512 + 512],
                                 start=False, stop=(a == 3))

            # H pass via stt (weights pre-scaled /16 on host).
            # stt can read only one PSUM operand, so land fyc' in SBUF first.
            STTV = nc.vector.scalar_tensor_tensor
            STTG = nc.gpsimd.scalar_tensor_tensor
            fyc_sb = sbP.tile([128, 544], F16, name="fyc_sb")
            nc.vector.tensor_scalar(fyc_sb[:], p_main[:], 1.0, t_by16,
                                    MUL, ADD)
            fyH = sbP.tile([128, 1024], F16, name="fyH")
            pmv = fyc_sb[:, 0:512].rearrange("q (s c) -> q s c", s=16)
            fyHe = fyH[:].rearrange("q (m e c) -> q m e c", m=16, e=2)
            STTV(fyHe[:, 1:16, 0, :], pmv[:, 1:16, :], 3.0,
                 pmv[:, 0:15, :], MUL, ADD)
            STTV(fyHe[:, 0:15, 1, :], pmv[:, 0:15, :], 3.0,
                 pmv[:, 1:16, :], MUL, ADD)
            nc.gpsimd.tensor_scalar(fyH[0:64, 0:32], fyc_sb[0:64, 0:32],
                                     4.0, 0.0, MUL, ADD)
            STTV(fyH[64:128, 0:32], fyc_sb[64:128, 0:32], 3.0,
                 fyc_sb[64:128, 512:544], MUL, ADD)
            STTV(fyH[0:64, 992:1024], fyc_sb[0:64, 480:512], 3.0,
                 fyc_sb[0:64, 512:544], MUL, ADD)
            nc.gpsimd.tensor_scalar(fyH[64:128, 992:1024],
                                     fyc_sb[64:128, 480:512], 4.0, 0.0,
                                     MUL, ADD)

            # W pass via stt on SBUF fp16: fy2p = 3*fyH'[k] + fyH'[k +- 1]
            # fy2p key order is E-MAJOR: col = r*64 + e*32 + k (fine w=2k+e)
            fyHv = fyH[:].rearrange("q (r k) -> q r k", r=32)
            fy2v = fy2p[:].rearrange("q (r e k) -> q r e k", r=32, e=2)
            for rs in (slice(0, 16), slice(16, 32)):
                STTV(fy2v[:, rs, 0, 1:32], fyHv[:, rs, 1:32], 3.0,
                     fyHv[:, rs, 0:31], MUL, ADD)
                STTV(fy2v[:, rs, 1, 0:31], fyHv[:, rs, 0:31], 3.0,
                     fyHv[:, rs, 1:32], MUL, ADD)
            nc.gpsimd.tensor_scalar(fy2v[:, :, 0, 0], fyHv[:, :, 0],
                                    4.0, 0.0, MUL, ADD)
            nc.vector.tensor_scalar(fy2v[:, :, 1, 31], fyHv[:, :, 31],
                                    4.0, 0.0, MUL, ADD)

            # ---- fx path: fx2 = [Wx12|Wx12] @ x + bx (fused) ----
            for blk in range(4):
                p = psP.tile([128, 512], F32, tag="pb", bufs=2,
                             name=f"p_fx2_{blk}")
                for a in range(2):
                    nc.tensor.matmul(
                        p[:], t_wx12[:, a * 128:(a + 1) * 128],
                        t_xl[:, blk * 1024 + a * 512:blk * 1024 + a * 512 + 512],
                        start=(a == 0), stop=(a == 1))
                if blk % 2 == 0:
                    nc.vector.tensor_scalar(
                        fx2[:, blk * 512:blk * 512 + 512], p[:], 1.0,
                        t_bx2d, MUL, ADD)
                else:
                    nc.scalar.activation(
                        fx2[:, blk * 512:blk * 512 + 512], p[:], IDENT,
                        bias=t_bx2d, scale=1.0)

            # ---- fself helpers ----
            def cp_dve(o, i):
                nc.vector.tensor_copy(o, i)

            def cp_act(o, i):
                nc.scalar.activation(o, i, COPY)

            # fselfT is PAIR-MAJOR: pair j = [Z, chunk j (64ch), Z,
            # chunk 16+j (64ch)] at cols 130j..130j+130.  Z columns are a
            # one-time memset; the fself bias is folded into b_up on host.
            zv = fselfT[:].rearrange("q (p c w) -> q p c w", p=16, c=2)
            nc.vector.memset(zv[:, :, :, 0], 1.0)

            # chunk j keys = xk block 2*(j//4) (top) cols (j%4)*128;
            # chunk 16+j = xk block 2*(j//4)+1 (bottom)
            def fs_pair(pool, tag, j, cp, nbufs=1):
                p = pool.tile([128, 1024], F32, tag=tag, bufs=nbufs,
                              name=f"p_fs_{j}")
                for g in range(2):
                    i = 2 * (j // 4) + g
                    k0 = i * 1024 + (j % 4) * 128
                    for a in range(2):
                        nc.tensor.matmul(
                            p[:, g * 512:g * 512 + 64],
                            t_xk[:, k0 + a * 512:k0 + a * 512 + 128],
                            t_ws12[:, a * 64:(a + 1) * 64],
                            start=(a == 0), stop=(a == 1))
                src = p[:].rearrange("q (c w) -> q c w", w=512)[:, :, 0:64]
                dst = fselfT[:, 130 * j:130 * j + 130].rearrange(
                    "q (c w) -> q c w", c=2)[:, :, 1:65]
                cp(dst, src)

            sbP_cm.__exit__(None, None, None)
            psP_cm.__exit__(None, None, None)

            # ================= phase 2: attention =====================
            fout_accs = {}

            def sim_unit(pool, h, u):
                p_, qb = u // 2, u % 2
                q0 = h * 1024 + qb * 512
                st = pool.tile([128, 1024], F32, tag="sim", bufs=3,
                               name=f"st_{h}_{u}")
                nc.tensor.matmul(st[:, 0:512],
                                 fy2p[0:64, p_ * 128:(p_ + 1) * 128],
                                 fx2[0:64, q0:q0 + 512],
                                 start=True, stop=True)
                nc.tensor.matmul(st[:, 512:1024],
                                 fy2p[64:128, p_ * 128:(p_ + 1) * 128],
                                 fx2[64:128, q0:q0 + 512],
                                 start=True, stop=True)
                return st

            def exp_unit(st, h, u, use_act):
                et = sbM.tile([128, 1024], BF16, tag="et", bufs=6,
                              name=f"et_{h}_{u}")
                if debug and h == 0 and u == 0:
                    d0 = sbM.tile([128, 1024], F32)
                    nc.vector.tensor_copy(d0[:], st[:])
                    nc.sync.dma_start(d_sim0[:], d0[:])
                if use_act:
                    nc.scalar.activation(et[:], st[:], EXP)
                else:
                    nc.vector.tensor_scalar(et[:].bitcast(I16), st[:],
                                            SCH_A, SCH_B, MUL, ADD)
                if debug and h == 0 and u == 0:
                    d1 = sbM.tile([128, 1024], F32)
                    nc.vector.tensor_copy(d1[:], et[:])
                    nc.sync.dma_start(d_et0[:], d1[:])
                return et

            def pv_unit(fout_acc, et, u):
                p_, qb = u // 2, u % 2
                cs = slice(qb * 512, (qb + 1) * 512)
                nc.tensor.matmul(fout_acc[:, cs],
                                 fselfT[:, 130 * p_:130 * p_ + 65],
                                 et[:, 0:512],
                                 start=(p_ == 0), stop=False)
                nc.tensor.matmul(fout_acc[:, cs],
                                 fselfT[:, 130 * p_ + 65:130 * p_ + 130],
                                 et[:, 512:1024],
                                 start=False, stop=(p_ == 15))

            def half_loop(h, psB, sched, hooks):
                fout_acc = fout_accs[h]
                sims = {0: sim_unit(psB, h, 0), 1: sim_unit(psB, h, 1)}
                for u in range(32):
                    if u + 2 < 32:
                        sims[u + 2] = sim_unit(psB, h, u + 2)
                    et = exp_unit(sims.pop(u), h, u, sched[u])
                    for fn in hooks.get(u, ()):
                        fn()
                    pv_unit(fout_acc, et, u)
                    if u == 30:
                        pre_tail_piece(h, 0)

            def pre_tail_piece(h, s):
                fout_acc = fout_accs[h]
                if debug and s == 0:
                    d3 = sbM.tile([65, 1024], F32, tag="dfout", bufs=2,
                                  name=f"dfout_{h}")
                    nc.vector.tensor_copy(d3[:, 0:512], fout_acc[:, 0:512])
                    nc.sync.dma_start(d_fout[:, h * 1024:h * 1024 + 512],
                                      d3[:, 0:512])
                cs = slice(s * 512, (s + 1) * 512)
                invz = sbM.tile([1, 512], F32, tag="zrow", bufs=2,
                                name=f"invz_{h}_{s}")
                nc.vector.reciprocal_approx_fast(invz[:], fout_acc[0:1, cs])
                invzb = sbM.tile([128, 512], F32, tag="izb", bufs=2,
                                 name=f"invzb_{h}_{s}")
                nc.gpsimd.partition_broadcast(invzb[:], invz[:])
                nc.vector.tensor_mul(
                    scaled[:, h * 1024 + s * 512:h * 1024 + (s + 1) * 512],
                    fout_acc[:, cs], invzb[0:65, :])

            def pre_tail(h):
                pre_tail_piece(h, 0)
                pre_tail_piece(h, 1)

            def up_item(psC, q, a, tag="up", nbufs=2, shape=512):
                def fn():
                    p = psC.tile([128, shape], F32, tag=tag, bufs=nbufs,
                                 name=f"p_up_{q}_{a}")
                    nc.tensor.matmul(p[:, 0:512],
                                     t_wupt[:, a * 128:(a + 1) * 128],
                                     scaled[:, q * 512:(q + 1) * 512],
                                     start=True, stop=True)
                    xlv = t_xl[:, q * 1024 + a * 512:
                               q * 1024 + a * 512 + 512]
                    cs = slice(a * 2048 + q * 512, a * 2048 + (q + 1) * 512)
                    nc.vector.tensor_add(out_sb[:, cs], p[:, 0:512], xlv)
                    nc.gpsimd.dma_start(out[:, cs], out_sb[:, cs])
                return fn

            with tc.tile_pool(name="psA", bufs=1, space="PSUM") as psA, \
                 tc.tile_pool(name="psB", bufs=1, space="PSUM") as psB:
                fout_accs[0] = psA.tile([65, 1024], F32, name="fout0")
                # fself preamble for pv units 0-3
                fs_pair(psB, "sim", 0, cp_act, nbufs=3)
                fs_pair(psB, "sim", 1, cp_dve, nbufs=3)
                hooks0 = {}
                fsj = {0: 2, 1: 3, 2: 4, 3: 5, 4: 6, 5: 7, 6: 8,
                       7: 9, 8: 10, 9: 11, 10: 12, 11: 13,
                       12: 14, 13: 15}
                for u_, j_ in fsj.items():
                    cp = cp_dve if j_ % 2 else cp_act
                    hooks0[u_] = [
                        (lambda jj=j_, cc=cp:
                         fs_pair(psB, "sim", jj, cc, nbufs=3))]
                half_loop(0, psB, sched0, hooks0)
                pre_tail_piece(0, 1)
                fout1_t = psB.tile([128, 1024], F32, tag="sim", bufs=3,
                                   name="fout1")
                fout_accs[1] = fout1_t[0:65, :]
                hooks1 = {
                    2: [up_item(psB, 0, 0, tag="sim", nbufs=3, shape=1024)],
                    5: [up_item(psB, 0, 1, tag="sim", nbufs=3, shape=1024)],
                    8: [up_item(psB, 1, 0, tag="sim", nbufs=3, shape=1024)],
                    11: [up_item(psB, 1, 1, tag="sim", nbufs=3, shape=1024)],
                    31: [up_item(psB, 2, 0, tag="sim", nbufs=3, shape=1024),
                         up_item(psB, 2, 1, tag="sim", nbufs=3, shape=1024)],
                }
                half_loop(1, psB, sched1, hooks1)
                pre_tail_piece(1, 1)
                for a in range(2):
                    up_item(psB, 3, a, tag="sim", nbufs=3, shape=1024)()

            if debug:
                d2 = sbM.tile([128, 2080], F32)
                nc.vector.tensor_copy(d2[:], fselfT[:])
                nc.sync.dma_start(d_fself[:], d2[:])
                for nm, src, shp, dst in (
                        ("d_fy2p", fy2p, [128, 2048], d_fy2p),
                        ("d_fx2", fx2, [128, 2048], d_fx2),
                        ("d_scaled", scaled, [65, 2048], d_scaled)):
                    dt_ = sbM.tile(shp, F32, name=f"dbg_{nm}")
                    nc.vector.tensor_copy(dt_[:], src[:])
                    nc.sync.dma_start(dst, dt_[:])


    nc.compile()
    return nc


def _prep_maps(x, y, W_self1, b_self1, W_self2, b_self2, W_x1, b_x1, W_x2,
               b_x2, W_y1, b_y1, W_y2, b_y2, W_up, b_up):
    f64 = np.float64

    def fold(W2, b1, b2):
        return W2.astype(f64) @ b1.astype(f64) + b2.astype(f64)

    def fuse(W2, W1):
        return W2.astype(f64) @ W1.astype(f64)

    ws12 = fuse(W_self2, W_self1)            # [64, 256]
    wx12 = fuse(W_x2, W_x1)                  # [64, 256]
    wy12 = fuse(W_y2, W_y1)                  # [64, 512]
    bs = fold(W_self2, b_self1, b_self2)     # folded into b_up
    bx = fold(W_x2, b_x1, b_x2)
    by = fold(W_y2, b_y1, b_y2)
    b_up2 = (W_up.astype(f64) @ bs + b_up.astype(f64))

    wp = np.zeros((128, 1666), np.float16)
    # ws12a: per cin chunk a, [128, 64] at cols a*64
    for a in range(2):
        wp[:, a * 64:(a + 1) * 64] = ws12.T[a * 128:(a + 1) * 128, :]
    # wx12a: per chunk a, [Wx12.T | Wx12.T] at cols 128 + a*128
    for a in range(2):
        blk = wx12.T[a * 128:(a + 1) * 128, :]
        wp[:, 128 + a * 128:128 + a * 128 + 64] = blk
        wp[:, 128 + a * 128 + 64:128 + (a + 1) * 128] = blk
    # wy12z: per cin chunk a (of 4): [W/16 | 0] then [0 | W/16]
    for a in range(4):
        blk = wy12.T[a * 128:(a + 1) * 128, :] / 16.0
        wp[:, 384 + (2 * a) * 128:384 + (2 * a) * 128 + 64] = blk
        wp[:, 384 + (2 * a + 1) * 128 + 64:384 + (2 * a + 2) * 128] = blk
    # wupt [65, 256] with fself bias folded into row 0
    wp[0, 1408:1664] = b_up2
    wp[1:65, 1408:1664] = W_up.T
    wb = np.zeros((128, 2), np.float32)
    wb[0:64, 0] = bx
    wb[64:128, 0] = bx
    wb[0:64, 1] = by / 16.0
    wb[64:128, 1] = by / 16.0

    maps = []
    perm = np.concatenate([np.arange(0, 64, 2), np.arange(1, 64, 2)])
    for b in range(B):
        xf = x[b].reshape(CX, N)
        yf = y[b].reshape(CY, HY * WY)
        yb_h = np.ascontiguousarray(
            yf.reshape(4, 128, 2, 512).transpose(1, 2, 0, 3)
            .reshape(128, 4096).astype(np.float16))
        halves = []
        for half in range(2):
            xh = xf[:, half * NH:(half + 1) * NH]
            halves.append(np.ascontiguousarray(
                xh.reshape(2, 128, 4, 512).transpose(1, 2, 0, 3)
                .reshape(128, 4096).astype(np.float16)))
        # xk: blocks interleaved (top b, bot b) in xl-block layout, with
        # E-MAJOR pixel order inside each block (w = 2k+e -> e*32+k)
        xk_h = np.empty((128, 8192), np.float16)
        for bb in range(4):
            for g in range(2):
                blkdat = halves[g][:, bb * 1024:(bb + 1) * 1024]
                v = blkdat.reshape(128, 2, 8, 64)[:, :, :, perm]
                xk_h[:, (2 * bb + g) * 1024:(2 * bb + g + 1) * 1024] = \
                    v.reshape(128, 1024)
        xk_h = np.ascontiguousarray(xk_h)
        for half in range(2):
            maps.append({
                "xl": halves[half], "xk": xk_h, "yb": yb_h, "wpack": wp,
                "wb32": wb,
            })
    return maps


def _run(inputs, trace=False, trace_kwargs=None, debug=False):
    key = ("nc", debug)
    if key not in _CACHE:
        _CACHE[key] = _build(debug=debug)
    nc = _CACHE[key]
    maps = _prep_maps(**inputs)
    res = run_bass_kernel_spmd(nc, maps, list(range(8)), trace=trace,
                               **(trace_kwargs or {}))
    outs = np.empty((B, CX, H, W), np.float32)
    for b in range(B):
        for half in range(2):
            o = res.results[2 * b + half]["out"]
            oh = o.reshape(128, 2, NH).transpose(1, 0, 2).reshape(CX, NH)
            outs[b, :, :, :].reshape(CX, N)[:, half * NH:(half + 1) * NH] = oh
    return outs, res


def kernel(**inputs):
    outs, _ = _run(inputs, trace=False)
    return outs
